# revision 1
# baseline (speedup 1.0000x reference)
"""Trainium2 Bass kernel for DenseDilatedKnnGraph (B=4, C=192, N=M=3136, K=9).

Computes, per batch: L2-normalize x,y over channels; dist = cdist(xn, yn) +
relative_pos; output the indices of the 9 smallest distances per query row,
stacked with the center indices -> (2, B, N, 9) int32.

Sharding: query rows (N) split across 8 NeuronCores (392 rows each); y and
relative_pos rows are read per core; indices into y are global so no gather
is needed.

Per-core pipeline (per batch b, row-tile t of 98 rows):
  PE:  psum = x^T yn via bf16 hi/lo split (6 accumulating matmuls at full
       rate: xh*yh, xl*yh, xh*yl across the two contraction halves) --
       ~1e-6-accurate, 4x faster than native fp32
  ACT: s = sqrt(psum * (-2/||x_row||) + 2)   [d^2 = 2 - 2*cos, x2=y2=1]
  DVE: neg = (s * -1) - relpos               [negated distance]
  DVE: top-9 via 4-segment max8/max_index + small merge + arithmetic
       index gather (one-hot dot products)
"""

import numpy as np

import concourse.bacc as bacc
import concourse.bass as bass
import concourse.mybir as mybir
import concourse.tile as tile
from concourse.bass_utils import run_bass_kernel_spmd

B, C, N, M, K = 4, 192, 3136, 3136, 9
NCORES = 8
NB = N // NCORES  # 392 rows per core
TR = 98           # rows per compute tile
NT = NB // TR     # 4 tiles per batch
C0, C1 = 128, 64  # contraction split of C=192
NSEG = 4          # top-k segments per row
W = M // NSEG     # 784
NC_ = NSEG * 8    # 32 merge candidates

# psum column chunks: bank-aligned (512 fp32 = one 2KB bank), 3136 = 6*512+64
CHUNKS = [(i * 512, min(512, M - i * 512)) for i in range((M + 511) // 512)]

F32 = mybir.dt.float32
F32R = mybir.dt.float32r
BF16 = mybir.dt.bfloat16
U32 = mybir.dt.uint32
I32 = mybir.dt.int32
NEG_BIG = -3.0e38
Alu = mybir.AluOpType


def _build_kernel():
    nc = bacc.Bacc("TRN2", target_bir_lowering=False, debug=False,
                   num_devices=NCORES)
    x_ap = nc.dram_tensor("x_blk", [B, C, NB], F32, kind="ExternalInput").ap()
    y_ap = nc.dram_tensor("y_full", [B, C, M], F32, kind="ExternalInput").ap()
    rp_ap = nc.dram_tensor("relpos", [NB, M], F32, kind="ExternalInput").ap()
    out_ap = nc.dram_tensor("out_idx", [B, NB, K], U32,
                            kind="ExternalOutput").ap()

    with tile.TileContext(nc) as tc:
        _emit(tc, out_ap, x_ap, y_ap, rp_ap)
    nc.compile()
    return nc


def _emit(tc, out_ap, x_ap, y_ap, rp_ap):
    nc = tc.nc
    from contextlib import ExitStack
    with ExitStack() as ctx:
        const_p = ctx.enter_context(tc.tile_pool(name="const", bufs=1))
        rp_p = ctx.enter_context(tc.tile_pool(name="rp", bufs=1))
        x_p = ctx.enter_context(tc.tile_pool(name="x", bufs=1))
        y_p = ctx.enter_context(tc.tile_pool(name="y", bufs=1))
        yh_p = ctx.enter_context(tc.tile_pool(name="yh", bufs=1))
        big_p = ctx.enter_context(tc.tile_pool(name="big", bufs=2))
        sm_p = ctx.enter_context(tc.tile_pool(name="sm", bufs=3))
        ps_p = ctx.enter_context(tc.tile_pool(name="ps", bufs=4, space="PSUM"))
        psn_p = ctx.enter_context(tc.tile_pool(name="psn", bufs=1, space="PSUM"))

        # ---- constants ----
        onesf = const_p.tile([128, 128], F32, tag="onesf")
        nc.vector.memset(onesf[:, :], 1.0)
        ones = const_p.tile([128, 128], F32R, tag="ones")
        nc.vector.tensor_copy(ones[:, :], onesf[:, :])
        two_col = const_p.tile([TR, 1], F32, tag="two")
        nc.vector.memset(two_col[:, :], 2.0)
        # iota over merge-candidate slots, and global segment offsets, as f32
        iotaf = const_p.tile([TR, NC_], F32, tag="iotaf")
        offsf = const_p.tile([TR, NC_], F32, tag="offsf")
        for c in range(NC_):
            nc.vector.memset(iotaf[:, c:c + 1], float(c))
        for s in range(NSEG):
            nc.vector.memset(offsf[:, 8 * s:8 * (s + 1)], float(W * s))

        # ---- persistent loads ----
        rp_sb = rp_p.tile([TR, NT, M], F32, tag="rp")
        nc.sync.dma_start(rp_sb[:, :, :],
                          rp_ap.rearrange("(t p) m -> p t m", p=TR))
        x0 = x_p.tile([C0, B, NB], F32, tag="x0")
        x1 = x_p.tile([C1, B, NB], F32, tag="x1")
        x_cbn = x_ap.rearrange("b c n -> c b n")
        nc.sync.dma_start(x0[:, :, :], x_cbn[0:C0])
        nc.sync.dma_start(x1[:, :, :], x_cbn[C0:C])
        x0f = x0[:, :, :].rearrange("c b n -> c (b n)")
        x1f = x1[:, :, :].rearrange("c b n -> c (b n)")

        # ---- x hi/lo split (bf16) ----
        xh0 = x_p.tile([C0, B * NB], BF16, tag="xh0")
        xh1 = x_p.tile([C1, B * NB], BF16, tag="xh1")
        xl0 = x_p.tile([C0, B * NB], BF16, tag="xl0")
        xl1 = x_p.tile([C1, B * NB], BF16, tag="xl1")
        nc.vector.tensor_copy(xh0[:, :], x0f)
        nc.vector.tensor_copy(xh1[:, :], x1f)
        nc.vector.scalar_tensor_tensor(xl0[:, :], x0f, 0.0, xh0[:, :],
                                       op0=Alu.bypass, op1=Alu.subtract)
        nc.vector.scalar_tensor_tensor(xl1[:, :], x1f, 0.0, xh1[:, :],
                                       op0=Alu.bypass, op1=Alu.subtract)

        # ---- x norms -> scale column -2/||x_row|| for all (b, t) ----
        sqx0 = big_p.tile([C0, B * NB], F32, tag="bigA")
        sqx1 = big_p.tile([C1, B * NB], F32, tag="bigB")
        nc.vector.scalar_tensor_tensor(sqx0[:, :], x0f, 0.0, x0f,
                                       op0=Alu.bypass, op1=Alu.mult)
        nc.vector.scalar_tensor_tensor(sqx1[:, :], x1f, 0.0, x1f,
                                       op0=Alu.bypass, op1=Alu.mult)
        sqx0r = sqx0[:, :]
        sqx1r = sqx1[:, :]
        nx2 = psn_p.tile([TR, B * NT], F32, tag="nx2")
        for b in range(B):
            for t in range(NT):
                j = b * NT + t
                lo = b * NB + t * TR
                nc.tensor.matmul(nx2[:, j:j + 1], sqx0r[:, lo:lo + TR],
                                 onesf[0:C0, 0:1], start=True, stop=False)
                nc.tensor.matmul(nx2[:, j:j + 1], sqx1r[:, lo:lo + TR],
                                 onesf[0:C1, 0:1], start=False, stop=True)
        nxr = const_p.tile([TR, B * NT], F32, tag="nxr")
        scale_col = const_p.tile([TR, B * NT], F32, tag="scale")
        for j in range(B * NT):
            nc.scalar.sqrt(nxr[:, j:j + 1], nx2[:, j:j + 1])
        nc.vector.reciprocal(nxr[:, :], nxr[:, :])
        nc.vector.tensor_scalar_mul(scale_col[:, :], nxr[:, :], -2.0)

        for b in range(B):
            # ---- normalize y (batch b) on device, then bf16 hi/lo split ----
            y0 = y_p.tile([C0, M], F32, tag="y0")
            y1 = y_p.tile([C1, M], F32, tag="y1")
            nc.sync.dma_start(y0[:, :], y_ap[b, 0:C0, :])
            nc.sync.dma_start(y1[:, :], y_ap[b, C0:C, :])
            sq0 = big_p.tile([C0, M], F32, tag="bigA")
            sq1 = big_p.tile([C1, M], F32, tag="bigB")
            nc.vector.scalar_tensor_tensor(sq0[:, :], y0[:, :], 0.0, y0[:, :],
                                           op0=Alu.bypass, op1=Alu.mult)
            nc.vector.scalar_tensor_tensor(sq1[:, :], y1[:, :], 0.0, y1[:, :],
                                           op0=Alu.bypass, op1=Alu.mult)
            sq0c = big_p.tile([C0, M], F32R, tag="bigA")
            sq1c = big_p.tile([C1, M], F32R, tag="bigB")
            nc.vector.tensor_copy(sq0c[:, :], sq0[:, :])
            nc.vector.tensor_copy(sq1c[:, :], sq1[:, :])
            sq0r = sq0c[:, :]
            sq1r = sq1c[:, :]
            ny = big_p.tile([128, M], F32, tag="bigA")
            for lo_c, sz in CHUNKS:
                cs = slice(lo_c, lo_c + sz)
                ss = ps_p.tile([128, 512], F32, tag="psmain")
                nc.tensor.matmul(ss[:, 0:sz], ones[0:C0, :], sq0r[:, cs],
                                 start=True, stop=False)
                nc.tensor.matmul(ss[:, 0:sz], ones[0:C1, 0:128], sq1r[:, cs],
                                 start=False, stop=True)
                nc.scalar.sqrt(ny[:, cs], ss[:, 0:sz])
            nyr = big_p.tile([128, M], F32, tag="bigB")
            nyscr = big_p.tile([128, M], F32, tag="bigB")
            nc.vector.reciprocal_approx_accurate(nyr[:, :], ny[:, :],
                                                 nyscr[:, :])
            # yn in-place into y tiles (y raw is dead afterwards)
            nc.vector.scalar_tensor_tensor(y0[:, :], y0[:, :], 0.0,
                                           nyr[0:C0, :],
                                           op0=Alu.bypass, op1=Alu.mult)
            nc.vector.scalar_tensor_tensor(y1[:, :], y1[:, :], 0.0,
                                           nyr[0:C1, :],
                                           op0=Alu.bypass, op1=Alu.mult)
            # bf16 hi/lo of yn (on gpsimd to offload DVE)
            yh0 = yh_p.tile([C0, M], BF16, tag="yh0")
            yh1 = yh_p.tile([C1, M], BF16, tag="yh1")
            yl0 = yh_p.tile([C0, M], BF16, tag="yl0")
            yl1 = yh_p.tile([C1, M], BF16, tag="yl1")
            nc.vector.tensor_copy(yh0[:, :], y0[:, :])
            nc.vector.tensor_copy(yh1[:, :], y1[:, :])
            nc.vector.scalar_tensor_tensor(yl0[:, :], y0[:, :], 0.0,
                                           yh0[:, :],
                                           op0=Alu.bypass, op1=Alu.subtract)
            nc.vector.scalar_tensor_tensor(yl1[:, :], y1[:, :], 0.0,
                                           yh1[:, :],
                                           op0=Alu.bypass, op1=Alu.subtract)

            # ---- main tiles ----
            for t in range(NT):
                j = b * NT + t
                lo = b * NB + t * TR
                s_t = big_p.tile([TR, M], F32, tag="bigA")
                for lo_c, sz in CHUNKS:
                    cs = slice(lo_c, lo_c + sz)
                    pd = ps_p.tile([TR, 512], F32, tag="psmain")
                    xs = slice(lo, lo + TR)
                    nc.tensor.matmul(pd[:, 0:sz], xh0[:, xs], yh0[:, cs],
                                     start=True, stop=False)
                    nc.tensor.matmul(pd[:, 0:sz], xh1[:, xs], yh1[:, cs],
                                     start=False, stop=False)
                    nc.tensor.matmul(pd[:, 0:sz], xl0[:, xs], yh0[:, cs],
                                     start=False, stop=False)
                    nc.tensor.matmul(pd[:, 0:sz], xl1[:, xs], yh1[:, cs],
                                     start=False, stop=False)
                    nc.tensor.matmul(pd[:, 0:sz], xh0[:, xs], yl0[:, cs],
                                     start=False, stop=False)
                    nc.tensor.matmul(pd[:, 0:sz], xh1[:, xs], yl1[:, cs],
                                     start=False, stop=True)
                    nc.scalar.activation(s_t[:, cs], pd[:, 0:sz],
                                         mybir.ActivationFunctionType.Sqrt,
                                         bias=two_col[:, :],
                                         scale=scale_col[:, j:j + 1])
                neg = big_p.tile([TR, M], F32, tag="bigB")
                nc.vector.scalar_tensor_tensor(
                    neg[:, :], s_t[:, :], -1.0, rp_sb[:, t, :],
                    op0=Alu.mult, op1=Alu.subtract)

                # ---- top-9 via full-row suite ----
                v8 = sm_p.tile([TR, 8], F32, tag="v8")
                i8 = sm_p.tile([TR, 8], U32, tag="i8")
                negr = big_p.tile([TR, M], F32, tag="bigB")
                v9 = sm_p.tile([TR, 8], F32, tag="v9")
                i9 = sm_p.tile([TR, 8], U32, tag="i9")
                nc.vector.max(out=v8[:, :], in_=neg[:, :])
                nc.vector.max_index(out=i8[:, :], in_max=v8[:, :],
                                    in_values=neg[:, :])
                nc.vector.match_replace(out=negr[:, :], in_to_replace=v8[:, :],
                                        in_values=neg[:, :],
                                        imm_value=NEG_BIG)
                nc.vector.max(out=v9[:, :], in_=negr[:, :])
                nc.vector.max_index(out=i9[:, :], in_max=v9[:, :],
                                    in_values=negr[:, :])
                iout = sm_p.tile([TR, K], U32, tag="iout")
                nc.vector.tensor_copy(iout[:, 0:8], i8[:, :])
                nc.vector.tensor_copy(iout[:, 8:9], i9[:, 0:1])
                rows = slice(t * TR, (t + 1) * TR)
                nc.sync.dma_start(out_ap[b, rows, :], iout[:, :])


_NC = None


def _get_nc():
    global _NC
    if _NC is None:
        _NC = _build_kernel()
    return _NC


def _run(inputs, trace=False, trace_kwargs=None):
    x = np.asarray(inputs["x"], dtype=np.float32)
    y = np.asarray(inputs["y"], dtype=np.float32)
    rp = np.asarray(inputs["relative_pos"], dtype=np.float32)
    assert x.shape == (B, C, N, 1) and y.shape == (B, C, M, 1)
    assert rp.shape == (1, N, M)

    y_full = np.ascontiguousarray(y[..., 0])
    in_maps = []
    for i in range(NCORES):
        sl = slice(i * NB, (i + 1) * NB)
        in_maps.append({
            "x_blk": np.ascontiguousarray(x[:, :, sl, 0]),
            "y_full": y_full,
            "relpos": np.ascontiguousarray(rp[0, sl, :]),
        })
    nc = _get_nc()
    kwargs = {}
    if trace:
        kwargs = dict(trace=True, trace_cores=list(range(NCORES)),
                      trace_kwargs=trace_kwargs or {})
    res = run_bass_kernel_spmd(nc, in_maps, core_ids=list(range(NCORES)),
                               **kwargs)
    nn = np.empty((B, N, K), dtype=np.int32)
    for i in range(NCORES):
        sl = slice(i * NB, (i + 1) * NB)
        nn[:, sl, :] = res.results[i]["out_idx"].view(np.int32)
    center = np.broadcast_to(np.arange(N, dtype=np.int32)[None, :, None],
                             (B, N, K))
    out = np.stack((nn, center), axis=0)
    return out, res


def kernel(**inputs):
    out, _ = _run(inputs, trace=False)
    return out



# revision 2
# speedup vs baseline: 1.0047x; 1.0047x over previous
"""Trainium2 Bass kernel v3 for DenseDilatedKnnGraph (B=4, C=192, N=M=3136, K=9).

Per batch: L2-normalize x,y over channels; dist = cdist(xn, yn) + relative_pos;
output indices of the 9 smallest distances per query row, stacked with center
indices -> (2, B, N, 9) int32.

Sharding: core i handles batch b=i//2, query-row half h=i%2 (1568 rows) as
13 row-tiles (12x128 + 1x32). Each core sees only its batch's y.

Host prep (numpy, cheap): negated relpos; x split hi/lo into bf16-/10-bit-
representable f32 stationaries (xh0, [xh1;xl1], xl0 - all f32r-safe since PE
rounds f32r operands to ~11 mantissa bits); y pre-normalized (yn rows 0-127
and the stacked [yn1;yn1]); per-row scales -2/||x_row||.

Device per tile:
  PE  : psum chunks = xh0.yn0 + pack.ynstk + xl0.yn0  (3 f32r matmuls/chunk)
  ACT : s = sqrt(psum * scale + 2)                    [d^2 = 2 - 2*cos]
  POOL: neg = rpn - s            (per 784-col segment)
  DVE : top-9 = 4-segment max8 -> 32-candidate merge -> 2x find_index8
"""

import numpy as np

import concourse.bacc as bacc
import concourse.mybir as mybir
import concourse.tile as tile
from concourse.bass_utils import run_bass_kernel_spmd

B, C, N, M, K = 4, 192, 3136, 3136, 9
NCORES = 8
NL = N // 2                      # 1568 query rows per core
C0, C1 = 128, 64
TILES = [(t * 128, 128) for t in range(12)] + [(1536, 32)]
NT = len(TILES)                  # 13
CHUNKS = [(i * 512, min(512, M - i * 512)) for i in range((M + 511) // 512)]
NSEG = 4
W = M // NSEG                    # 784

F32 = mybir.dt.float32
F32R = mybir.dt.float32r
U32 = mybir.dt.uint32
NEG_BIG = -3.0e38
Alu = mybir.AluOpType
AF = mybir.ActivationFunctionType


def _build_kernel():
    nc = bacc.Bacc("TRN2", target_bir_lowering=False, debug=False,
                   num_devices=NCORES)
    xh0_ap = nc.dram_tensor("xh0", [C0, NL], F32R, kind="ExternalInput").ap()
    pack_ap = nc.dram_tensor("pack", [128, NL], F32R,
                             kind="ExternalInput").ap()
    xl0_ap = nc.dram_tensor("xl0", [C0, NL], F32R, kind="ExternalInput").ap()
    yn0_ap = nc.dram_tensor("yn0f", [C0, M], F32, kind="ExternalInput").ap()
    ystk_ap = nc.dram_tensor("ystkf", [128, M], F32,
                             kind="ExternalInput").ap()
    xsc_ap = nc.dram_tensor("xscale", [128, NT], F32,
                            kind="ExternalInput").ap()
    rpn_ap = nc.dram_tensor("rpn", [NL, M], F32, kind="ExternalInput").ap()
    out_ap = nc.dram_tensor("out_idx", [NL, K], U32,
                            kind="ExternalOutput").ap()
    with tile.TileContext(nc) as tc:
        _emit(tc, out_ap, xh0_ap, pack_ap, xl0_ap, yn0_ap, ystk_ap, xsc_ap,
              rpn_ap)
    nc.compile()
    return nc


def _emit(tc, out_ap, xh0_ap, pack_ap, xl0_ap, yn0_ap, ystk_ap, xsc_ap,
          rpn_ap):
    nc = tc.nc
    from contextlib import ExitStack
    with ExitStack() as ctx:
        const_p = ctx.enter_context(tc.tile_pool(name="const", bufs=1))
        x_p = ctx.enter_context(tc.tile_pool(name="x", bufs=1))
        y_p = ctx.enter_context(tc.tile_pool(name="y", bufs=1))
        rp_p = ctx.enter_context(tc.tile_pool(name="rp", bufs=3))
        st_p = ctx.enter_context(tc.tile_pool(name="st", bufs=1))
        sm_p = ctx.enter_context(tc.tile_pool(name="sm", bufs=2))
        ps_p = ctx.enter_context(tc.tile_pool(name="ps", bufs=1, space="PSUM"))

        # ---- constants ----
        two_col = const_p.tile([128, 2], F32, tag="two")
        nc.vector.memset(two_col[:, :], 2.0)
        scale_col = const_p.tile([128, NT], F32, tag="scale")
        nc.sync.dma_start(scale_col[:, :], xsc_ap)

        # ---- x stationaries (pre-split on host, f32r-ready) ----
        # column-chunked DMAs so tile 0 only waits on its first piece
        xh0 = x_p.tile([C0, NL], F32R, tag="xh0")
        pack = x_p.tile([128, NL], F32R, tag="pack")
        xl0 = x_p.tile([C0, NL], F32R, tag="xl0")
        PIECE = NL // 4
        def stat_dma(p):
            ps_ = slice(p * PIECE, (p + 1) * PIECE)
            nc.sync.dma_start(xh0[:, ps_], xh0_ap[:, ps_])
            nc.sync.dma_start(pack[:, ps_], pack_ap[:, ps_])
            nc.sync.dma_start(xl0[:, ps_], xl0_ap[:, ps_])
        stat_dma(0)

        def rp_dma(t):
            # segment-aligned quarter DMAs: pool seg s waits only on its piece
            lo, rows = TILES[t]
            xs = slice(lo, lo + rows)
            rpt = rp_p.tile([128, M], F32, tag="rp", name=f"rpt{t}")
            for s in range(NSEG):
                sg = slice(s * W, (s + 1) * W)
                nc.sync.dma_start(rpt[0:rows, sg], rpn_ap[xs, sg])
            return rpt

        # ---- y moving operands: DMA f32 staging, cast to f32r per chunk ----
        yn0f = y_p.tile([C0, M], F32, tag="yn0f")
        ystkf = y_p.tile([128, M], F32, tag="ystkf")
        yn0 = y_p.tile([C0, M], F32R, tag="yn0")
        ynstk = y_p.tile([128, M], F32R, tag="ynstk")
        for lo_c, sz in CHUNKS:
            cs = slice(lo_c, lo_c + sz)
            nc.sync.dma_start(yn0f[:, cs], yn0_ap[:, cs])
            nc.sync.dma_start(ystkf[:, cs], ystk_ap[:, cs])
            nc.vector.tensor_copy(yn0[:, cs], yn0f[:, cs])
            nc.vector.tensor_copy(ynstk[:, cs], ystkf[:, cs])
        rp_q = [rp_dma(0), rp_dma(1)]
        for p in range(1, 4):
            stat_dma(p)

        # ---- main loop over 13 row-tiles ----
        for t, (lo, rows) in enumerate(TILES):
            xs = slice(lo, lo + rows)
            rpt = rp_q.pop(0)
            if t + 2 < NT:
                rp_q.append(rp_dma(t + 2))
            s_t = st_p.tile([128, M], F32, tag="s", bufs=2)
            neg = st_p.tile([128, M], F32, tag="neg", bufs=3)
            for lo_c, sz in CHUNKS:
                cs = slice(lo_c, lo_c + sz)
                pd = ps_p.tile([128, 512], F32, tag="pd", bufs=6)
                nc.tensor.matmul(pd[0:rows, 0:sz], xh0[:, xs],
                                 yn0[:, cs], start=True, stop=False)
                nc.tensor.matmul(pd[0:rows, 0:sz], pack[:, xs],
                                 ynstk[:, cs], start=False, stop=False)
                nc.tensor.matmul(pd[0:rows, 0:sz], xl0[:, xs],
                                 yn0[:, cs], start=False, stop=True)
                nc.scalar.activation(s_t[0:rows, cs], pd[0:rows, 0:sz],
                                     AF.Sqrt, bias=two_col[0:rows, 0:1],
                                     scale=scale_col[0:rows, t:t + 1])

            # ---- top-9: segmented max8 + merge + find_index8 x2 ----
            v32 = sm_p.tile([128, 32], F32, tag="v32")
            for s in range(NSEG):
                sg = slice(s * W, (s + 1) * W)
                nc.gpsimd.tensor_tensor(neg[0:rows, sg], rpt[0:rows, sg],
                                        s_t[0:rows, sg], op=Alu.subtract)
                nc.vector.max(v32[0:rows, 8 * s:8 * s + 8],
                              neg[0:rows, sg])
            m8 = sm_p.tile([128, 8], F32, tag="m8")
            nc.vector.max(m8[0:rows, :], v32[0:rows, :])
            v32r = sm_p.tile([128, 32], F32, tag="v32r")
            nc.vector.match_replace(v32r[0:rows, :],
                                    in_to_replace=m8[0:rows, :],
                                    in_values=v32[0:rows, :],
                                    imm_value=NEG_BIG)
            v9 = sm_p.tile([128, 8], F32, tag="v9")
            nc.vector.max(v9[0:rows, :], v32r[0:rows, :])
            iout = sm_p.tile([128, 16], U32, tag="iout")
            nc.vector.max_index(iout[0:rows, 0:8], m8[0:rows, :],
                                neg[0:rows, :])
            i9 = sm_p.tile([128, 8], U32, tag="i9")
            nc.vector.max_index(i9[0:rows, :], v9[0:rows, :], neg[0:rows, :])
            nc.vector.tensor_copy(iout[0:rows, 8:9], i9[0:rows, 0:1])
            nc.sync.dma_start(out_ap[xs, :], iout[0:rows, 0:K])


def _rne_mask(u, drop):
    """Round-to-nearest-even float32 bit-trick, dropping `drop` mantissa bits."""
    half = np.uint32((1 << (drop - 1)) - 1)
    lsb = (u >> np.uint32(drop)) & np.uint32(1)
    return (u + half + lsb) & np.uint32((~((1 << drop) - 1)) & 0xFFFFFFFF)


_NC = None


def _get_nc():
    global _NC
    if _NC is None:
        _NC = _build_kernel()
    return _NC


def _prep(x, y, rp):
    """Host-side prep: hi/lo split of x, normalized y, scales, negated rp."""
    xsq = x.astype(np.float64)
    nx = np.sqrt((xsq * xsq).sum(axis=1))          # (B, N)
    u = x.view(np.uint32)
    xh = _rne_mask(u, 16).view(np.float32)         # bf16-representable
    xl = x - xh
    xl10 = _rne_mask(xl.view(np.uint32), 13).view(np.float32)
    ysq = y.astype(np.float64)
    ny = np.sqrt((ysq * ysq).sum(axis=1))          # (B, M)
    yn = (y / np.maximum(ny, 1e-12)[:, None, :]).astype(np.float32)
    xscale = (-2.0 / np.maximum(nx, 1e-12)).astype(np.float32)  # (B, N)
    rpn = -rp[0]
    return xh, xl10, yn, xscale, rpn


def _run(inputs, trace=False, trace_kwargs=None):
    x = np.ascontiguousarray(np.asarray(inputs["x"], dtype=np.float32)[..., 0])
    y = np.ascontiguousarray(np.asarray(inputs["y"], dtype=np.float32)[..., 0])
    rp = np.asarray(inputs["relative_pos"], dtype=np.float32)
    assert x.shape == (B, C, N) and y.shape == (B, C, M)
    assert rp.shape == (1, N, M)

    xh, xl10, yn, xscale, rpn = _prep(x, y, rp)
    in_maps = []
    for i in range(NCORES):
        b, h = i // 2, i % 2
        sl = slice(h * NL, (h + 1) * NL)
        xsc_t = np.zeros((128, NT), dtype=np.float32)
        sc = xscale[b, sl]                         # (NL,)
        for t, (lo, rows) in enumerate(TILES):
            xsc_t[0:rows, t] = sc[lo:lo + rows]
        in_maps.append({
            "xh0": np.ascontiguousarray(xh[b, 0:C0, sl]),
            "pack": np.ascontiguousarray(
                np.concatenate([xh[b, C0:C, sl], xl10[b, C0:C, sl]], axis=0)),
            "xl0": np.ascontiguousarray(xl10[b, 0:C0, sl]),
            "yn0f": np.ascontiguousarray(yn[b, 0:C0, :]),
            "ystkf": np.ascontiguousarray(
                np.concatenate([yn[b, C0:C, :], yn[b, C0:C, :]], axis=0)),
            "xscale": xsc_t,
            "rpn": np.ascontiguousarray(rpn[sl, :]),
        })
    nc = _get_nc()
    kwargs = {}
    if trace:
        kwargs = dict(trace=True, trace_cores=list(range(NCORES)),
                      trace_kwargs=trace_kwargs or {})
    res = run_bass_kernel_spmd(nc, in_maps, core_ids=list(range(NCORES)),
                               **kwargs)
    nn = np.empty((B, N, K), dtype=np.int32)
    for i in range(NCORES):
        b, h = i // 2, i % 2
        sl = slice(h * NL, (h + 1) * NL)
        nn[b, sl, :] = res.results[i]["out_idx"].view(np.int32)
    center = np.broadcast_to(np.arange(N, dtype=np.int32)[None, :, None],
                             (B, N, K))
    out = np.stack((nn, center), axis=0)
    return out, res


def kernel(**inputs):
    out, _ = _run(inputs, trace=False)
    return out
